# revision 17
# baseline (speedup 1.0000x reference)
"""Trainium2 Bass kernel for nn_AttnDecoder (single-token attention decoder step).

Computation (the attention branch in the reference is dead code -- its result
never reaches an output -- so it is skipped):
    e      = emb[tok]                                   (host gather, 4 KB)
    gates  = W_ih @ e + b_ih + W_hh @ h0 + b_hh         (LSTM cell, torch gate order)
    h', c' = LSTM(gates, c0)
    logits = out_W @ h' + out_b
    out    = (log_softmax(logits), h', c')

Sharding (8 NeuronCores). Collectives on this fabric cost ~27us each
regardless of size, so the design uses exactly ONE:
    - The 4H gate dim of W_ih/W_hh is sharded: core k computes the H-slice
      [k*128:(k+1)*128) of all four gates, hence of h'/c'.
    - out_W is sharded by the CONTRACTION dim: core k holds the columns for
      its own h-slice and computes full-vocab partial logits with no h
      exchange; one 200KB AllReduce then sums the partials, and every core
      finishes the log_softmax locally in a [128, 400] layout.

All matvecs use the moving-weights orientation (activation chunk [128,1]
stationary, weights stream as the moving operand, bf16 at 1 cycle/row).
Partial logits land on PSUM partition rows {0,32,64,96} via the matmul
base-partition placement so they can be staged out 4 rows per copy.
"""

import numpy as np
import ml_dtypes

import concourse.bacc as bacc
import concourse.mybir as mybir
import concourse.tile as tile
from concourse import bass_utils

P = 128
H = 1024
V = 50000
NCORES = 8
KC = H // P            # contraction chunks of 128
GATES = 4
GF = GATES * P         # 512 gate rows per core
VFULL = 51200          # vocab padded to 100 * 512 = 128 * 400
TQ = VFULL // P        # 400
NB = 512               # moving-N per matmul (= one f32 PSUM bank)
NBLK = VFULL // NB     # 100 matmul blocks
QS = 4                 # blocks per PSUM tile, at partition rows {0,32,64,96}
NPT = NBLK // QS       # 25 psum tiles
SW = 8192              # vocab columns per weight-stripe DMA
PAD_BIAS = -80.0       # pad logits: exp(-80) == 0 vs sum ~1e5, dropped on host

TA = 13                # psum tiles in AllReduce chunk A (rest go in chunk B)
TQA = TA * QS * NB // P        # 192 columns of the [128, *] chunk-A view

F32 = mybir.dt.float32
BF16 = mybir.dt.bfloat16
FP16 = mybir.dt.float16

W_DT = BF16            # out_W stream dtype
LSTM_DT = BF16         # W_ih/W_hh stream dtype
AR_DT = FP16           # partial-logit AllReduce payload dtype

_np_dt = {F32: np.float32, BF16: ml_dtypes.bfloat16, FP16: np.float16}


def _emit(tc, io):
    nc = tc.nc
    AF = mybir.ActivationFunctionType
    ALU = mybir.AluOpType
    RG = [list(range(NCORES))]

    with (
        tc.tile_pool(name="iop", bufs=1) as iop,
        tc.tile_pool(name="wp", bufs=3) as wp,
        tc.tile_pool(name="ppg", bufs=1, space="PSUM") as ppg,
        tc.tile_pool(name="ppn", bufs=5, space="PSUM") as ppn,
        tc.tile_pool(name="dp", bufs=1, space="DRAM") as dp,
    ):
        # Warm the ACT LUTs so table loads stay off the critical path.
        warm = iop.tile([1, 1], F32)
        nc.vector.memset(warm[:, :], 0.0)
        for fn in (AF.Exp, AF.Ln, AF.Sigmoid, AF.Tanh):
            nc.scalar.activation(warm[:, :], warm[:, :], fn)

        # LSTM weights first (they gate the whole pipeline), then the packed
        # small inputs: two DMAs instead of five.
        wi_sb = iop.tile([P, KC, GF], LSTM_DT)
        wh_sb = iop.tile([P, KC, GF], LSTM_DT)
        nc.sync.dma_start(wi_sb[:, :, :], io["w_iT"][:, :].rearrange("(a p) g -> p a g", p=P))
        nc.sync.dma_start(wh_sb[:, :, :], io["w_hT"][:, :].rearrange("(a p) g -> p a g", p=P))
        eh_sb = iop.tile([P, 2 * KC], F32)          # e | h0
        nc.sync.dma_start(eh_sb[:, :], io["eh_in"][:, :])
        cb_sb = iop.tile([1, P + GF], F32)          # c0 | gate bias
        nc.sync.dma_start(cb_sb[:, :], io["cb_in"][:, :])
        b_sb = iop.tile([P, TQ], F32)
        nc.sync.dma_start(b_sb[:, :], io["b_full"][:, :])
        if LSTM_DT != F32:
            eh_mm = iop.tile([P, 2 * KC], LSTM_DT)
            nc.vector.tensor_copy(eh_mm[:, :], eh_sb[:, :])
        else:
            eh_mm = eh_sb

        # ---- LSTM gates on one partition: psum_gate[0, g*128+j] ----
        psum_gate = ppg.tile([1, GF], F32)
        for c in range(KC):
            nc.tensor.matmul(psum_gate[:, :], lhsT=eh_mm[:, c:c + 1],
                             rhs=wi_sb[:, c, :], start=(c == 0), stop=False)
        for c in range(KC):
            nc.tensor.matmul(psum_gate[:, :], lhsT=eh_mm[:, KC + c:KC + c + 1],
                             rhs=wh_sb[:, c, :], start=False, stop=(c == KC - 1))
        gsum = iop.tile([1, GF], F32)
        nc.vector.tensor_add(gsum[:, :], psum_gate[:, :], cb_sb[:, P:])
        gact = iop.tile([1, GF], F32)
        nc.scalar.activation(gact[:, 0:2 * P], gsum[:, 0:2 * P], AF.Sigmoid)       # i, f
        nc.scalar.activation(gact[:, 2 * P:3 * P], gsum[:, 2 * P:3 * P], AF.Tanh)  # g
        nc.scalar.activation(gact[:, 3 * P:4 * P], gsum[:, 3 * P:4 * P], AF.Sigmoid)  # o
        fc = iop.tile([1, P], F32)
        nc.vector.tensor_mul(fc[:, :], gact[:, P:2 * P], cb_sb[:, 0:P])
        ig = iop.tile([1, P], F32)
        nc.vector.tensor_mul(ig[:, :], gact[:, 0:P], gact[:, 2 * P:3 * P])
        c_new = iop.tile([1, P], F32)
        nc.vector.tensor_add(c_new[:, :], fc[:, :], ig[:, :])
        tanh_c = iop.tile([1, P], F32)
        nc.scalar.activation(tanh_c[:, :], c_new[:, :], AF.Tanh)
        h_new = iop.tile([1, P], F32)
        nc.vector.tensor_mul(h_new[:, :], gact[:, 3 * P:4 * P], tanh_c[:, :])

        nc.scalar.dma_start(io["h_out"][:, :], h_new[:, :])
        nc.scalar.dma_start(io["c_out"][:, :], c_new[:, :])

        # h' slice to stationary layout [128, 1] via a DRAM round-trip.
        hd = dp.tile([1, P], F32)
        nc.gpsimd.dma_start(hd[:, :], h_new[:, :])
        h_col = iop.tile([P, 1], F32)
        nc.gpsimd.dma_start(h_col[:, :], hd[:, :].rearrange("x p -> p x"))
        # h in column 0 of a [128, 32] stationary tile, zeros elsewhere: each
        # matmul then fills 32 PSUM rows (1 real + 31 zero), so 4 matmuls at
        # col_grp positions {0,32,64,96} initialize the whole [128, 512] tile
        # and it can be staged out with one plain full-tile copy.
        h_pad = iop.tile([P, 32], W_DT)
        nc.vector.memset(h_pad[:, :], 0.0)
        nc.vector.tensor_copy(h_pad[:, 0:1], h_col[:, :])

        # ---- full-vocab partial logits from this core's h-slice ----
        # Block j covers vocab [j*512, (j+1)*512); psum tile t=j//4 holds its
        # 4 blocks on partition rows {0,32,64,96}; the fp16 stage mirrors that.
        # Partials ship in two AllReduce chunks so the first one overlaps the
        # second half of the matvec (each collective has a ~20us floor here).
        stages = [iop.tile([P, TA, NB], AR_DT, name="stageA"),
                  iop.tile([P, NPT - TA, NB], AR_DT, name="stageB")]
        arins = [dp.tile([1, TA * QS * NB], AR_DT, name="arinA"),
                 dp.tile([1, (NPT - TA) * QS * NB], AR_DT, name="arinB")]
        arouts = [dp.tile([1, TA * QS * NB], AR_DT, name="aroutA"),
                  dp.tile([1, (NPT - TA) * QS * NB], AR_DT, name="aroutB")]
        for t in range(NPT):
            stripe = (t * QS) // (SW // NB)
            if (t * QS) % (SW // NB) == 0:
                w0 = stripe * SW
                bw = min(SW, VFULL - w0)
                wt = wp.tile([P, SW], W_DT, tag="wt")
                nc.sync.dma_start(wt[:, :bw], io["w_oT"][:, w0:w0 + bw])
            psum_n = ppn.tile([P, NB], F32, tag="pn")
            for q in range(QS):
                col = (t * QS + q) * NB - stripe * SW
                nc.tensor.matmul(psum_n[q * 32:(q + 1) * 32, :],
                                 lhsT=h_pad[:, :], rhs=wt[:, col:col + NB],
                                 start=True, stop=True,
                                 tile_position=(0, q * 32))
            ch, tt = (0, t) if t < TA else (1, t - TA)
            nc.vector.tensor_copy(stages[ch][:, tt, :], psum_n[:, :])
            if t == TA - 1 or t == NPT - 1:
                nt = TA if ch == 0 else NPT - TA
                for q in range(QS):
                    nc.gpsimd.dma_start(
                        arins[ch][:, :].rearrange("x (t q v) -> x t q v",
                                                  q=QS, v=NB)[:, :, q, :],
                        stages[ch][q * 32:q * 32 + 1, :, :],
                    )
                nc.gpsimd.collective_compute(
                    "AllReduce", ALU.add, replica_groups=RG,
                    ins=[arins[ch].opt()], outs=[arouts[ch].opt()],
                )

        # ---- replicated log_softmax epilogue, chunk c viewed [128, VC/128] ----
        s_parts = iop.tile([P, 2], F32)
        logits_ch = []
        for ch, (tq0, tqn) in enumerate(((0, TQA), (TQA, TQ - TQA))):
            ar_sb = iop.tile([P, tqn], AR_DT, name=f"arsb{ch}")
            nc.gpsimd.dma_start(ar_sb[:, :],
                              arouts[ch][:, :].rearrange("x (p t) -> (x p) t", p=P))
            logits_sb = iop.tile([P, tqn], F32, name=f"lg{ch}")
            nc.vector.tensor_add(logits_sb[:, :], ar_sb[:, :], b_sb[:, tq0:tq0 + tqn])
            # Logits are bounded (~|12|) for this model: exp needs no max-shift.
            exp_sb = iop.tile([P, tqn], F32, name=f"ex{ch}")
            nc.scalar.activation(exp_sb[:, :], logits_sb[:, :], AF.Exp,
                                 accum_out=s_parts[:, ch:ch + 1])
            logits_ch.append(logits_sb)
        s_part = iop.tile([P, 1], F32)
        nc.vector.reduce_sum(s_part[:, :], s_parts[:, :], axis=mybir.AxisListType.X)
        s_red = iop.tile([P, 1], F32)
        nc.gpsimd.partition_all_reduce(s_red[:, :], s_part[:, :], channels=P,
                                       reduce_op=bass_isa_reduce_add())
        logS = iop.tile([P, 1], F32)
        nc.scalar.activation(logS[:, :], s_red[:, :], AF.Ln)
        for ch, (tq0, tqn) in enumerate(((0, TQA), (TQA, TQ - TQA))):
            lp_sb = iop.tile([P, tqn], F32, name=f"lp{ch}")
            nc.vector.tensor_scalar_sub(lp_sb[:, :], logits_ch[ch][:, :], logS[:, :])
            nc.scalar.dma_start(io["lp_out"][:, tq0:tq0 + tqn], lp_sb[:, :])


def bass_isa_reduce_add():
    from concourse import bass_isa
    return bass_isa.ReduceOp.add


_cache = {}


def _build_nc():
    nc = bacc.Bacc("TRN2", target_bir_lowering=False, debug=False, num_devices=NCORES)
    io = {}
    for name, shape, dt in [
        ("w_oT", [P, VFULL], W_DT), ("b_full", [P, TQ], F32),
        ("w_iT", [H, GF], LSTM_DT), ("w_hT", [H, GF], LSTM_DT),
        ("eh_in", [P, 2 * KC], F32), ("cb_in", [1, P + GF], F32),
    ]:
        io[name] = nc.dram_tensor(name, shape, dt, kind="ExternalInput")
    for name, shape in [("lp_out", [P, TQ]), ("h_out", [1, P]), ("c_out", [1, P])]:
        io[name] = nc.dram_tensor(name, shape, F32, kind="ExternalOutput")

    with tile.TileContext(nc) as tc:
        _emit(tc, io)
    nc.compile()
    return nc


def _prep_inputs(inputs):
    emb = np.asarray(inputs["emb"], np.float32)
    tok = int(np.asarray(inputs["input_tok"]).ravel()[0])
    e = emb[tok]
    h0 = np.asarray(inputs["h0"], np.float32).reshape(H)
    c0 = np.asarray(inputs["c0"], np.float32).reshape(H)
    W_ih = np.asarray(inputs["W_ih"], np.float32)
    W_hh = np.asarray(inputs["W_hh"], np.float32)
    b = np.asarray(inputs["b_ih"], np.float32) + np.asarray(inputs["b_hh"], np.float32)
    out_W = np.asarray(inputs["out_W"], np.float32)
    out_b = np.asarray(inputs["out_b"], np.float32)

    w_np = _np_dt[W_DT]
    l_np = _np_dt[LSTM_DT]
    WT = np.ascontiguousarray(out_W.astype(w_np).T)       # [H, V]
    b_flat = np.full((VFULL,), PAD_BIAS, np.float32)
    b_flat[:V] = out_b
    VA = TQA * P
    b_full = np.concatenate([b_flat[:VA].reshape(P, TQA),
                             b_flat[VA:].reshape(P, TQ - TQA)], axis=1)
    b_full = np.ascontiguousarray(b_full)
    eh = np.concatenate([e.reshape(KC, P).T, h0.reshape(KC, P).T], axis=1)
    eh = np.ascontiguousarray(eh)

    in_maps = []
    for k in range(NCORES):
        rows = np.concatenate([np.arange(g * H + k * P, g * H + (k + 1) * P) for g in range(GATES)])
        w_iT = np.ascontiguousarray(W_ih[rows].T.astype(l_np))
        w_hT = np.ascontiguousarray(W_hh[rows].T.astype(l_np))
        w_oT = np.zeros((P, VFULL), w_np)
        w_oT[:, :V] = WT[k * P:(k + 1) * P, :]
        cb = np.concatenate([c0[k * P:(k + 1) * P], b[rows]]).reshape(1, P + GF)
        in_maps.append(dict(w_oT=w_oT, b_full=b_full, w_iT=w_iT, w_hT=w_hT,
                            eh_in=eh, cb_in=np.ascontiguousarray(cb)))
    return in_maps


def _run(inputs, trace=False, **kw):
    if "nc" not in _cache:
        _cache["nc"] = _build_nc()
    nc = _cache["nc"]
    in_maps = _prep_inputs(inputs)
    res = bass_utils.run_bass_kernel_spmd(nc, in_maps, core_ids=list(range(NCORES)),
                                          trace=trace, **kw)
    h_new = np.empty(H, np.float32)
    c_new = np.empty(H, np.float32)
    for k in range(NCORES):
        r = res.results[k]
        h_new[k * P:(k + 1) * P] = r["h_out"].reshape(-1)
        c_new[k * P:(k + 1) * P] = r["c_out"].reshape(-1)
    r0 = res.results[0]["lp_out"]
    lp = np.concatenate([np.ascontiguousarray(r0[:, :TQA]).reshape(-1),
                         np.ascontiguousarray(r0[:, TQA:]).reshape(-1)])[:V]
    out = (lp[None, :], h_new[None, None, :], c_new[None, None, :])
    return out, res


def kernel(**inputs):
    out, _ = _run(inputs)
    return out


# revision 18
# speedup vs baseline: 1.3500x; 1.3500x over previous
"""Trainium2 Bass kernel for nn_AttnDecoder (single-token attention decoder step).

Computation (the attention branch in the reference is dead code -- its result
never reaches an output -- so it is skipped):
    e      = emb[tok]                                   (host gather, 4 KB)
    gates  = W_ih @ e + b_ih + W_hh @ h0 + b_hh         (LSTM cell, torch gate order)
    h', c' = LSTM(gates, c0)
    logits = out_W @ h' + out_b
    out    = (log_softmax(logits), h', c')

Sharding (8 NeuronCores). Collectives on this fabric cost ~27us each
regardless of size, so the design uses exactly ONE:
    - The 4H gate dim of W_ih/W_hh is sharded: core k computes the H-slice
      [k*128:(k+1)*128) of all four gates, hence of h'/c'.
    - out_W is sharded by the CONTRACTION dim: core k holds the columns for
      its own h-slice and computes full-vocab partial logits with no h
      exchange; one 200KB AllReduce then sums the partials, and every core
      finishes the log_softmax locally in a [128, 400] layout.

All matvecs use the moving-weights orientation (activation chunk [128,1]
stationary, weights stream as the moving operand, bf16 at 1 cycle/row).
Partial logits land on PSUM partition rows {0,32,64,96} via the matmul
base-partition placement so they can be staged out 4 rows per copy.
"""

import numpy as np
import ml_dtypes

import concourse.bacc as bacc
import concourse.mybir as mybir
import concourse.tile as tile
from concourse import bass_utils

P = 128
H = 1024
V = 50000
NCORES = 8
KC = H // P            # contraction chunks of 128
GATES = 4
GF = GATES * P         # 512 gate rows per core
VFULL = 51200          # vocab padded to 100 * 512 = 128 * 400
TQ = VFULL // P        # 400
NB = 512               # moving-N per matmul (= one f32 PSUM bank)
NBLK = VFULL // NB     # 100 matmul blocks
QS = 4                 # blocks per PSUM tile, at partition rows {0,32,64,96}
NPT = NBLK // QS       # 25 psum tiles
SW = 8192              # vocab columns per weight-stripe DMA
PAD_BIAS = -80.0       # pad logits: exp(-80) == 0 vs sum ~1e5, dropped on host

TA = 13                # psum tiles in AllReduce chunk A (rest go in chunk B)
TQA = TA * QS * NB // P        # 192 columns of the [128, *] chunk-A view

F32 = mybir.dt.float32
BF16 = mybir.dt.bfloat16
FP16 = mybir.dt.float16
FP8 = mybir.dt.float8e4

W_DT = FP8             # out_W stream dtype
LSTM_DT = BF16         # W_ih/W_hh stream dtype
AR_DT = FP16           # partial-logit AllReduce payload dtype

_np_dt = {F32: np.float32, BF16: ml_dtypes.bfloat16, FP16: np.float16,
          FP8: ml_dtypes.float8_e4m3}


def _emit(tc, io):
    nc = tc.nc
    AF = mybir.ActivationFunctionType
    ALU = mybir.AluOpType
    RG = [list(range(NCORES))]

    with (
        tc.tile_pool(name="iop", bufs=1) as iop,
        tc.tile_pool(name="wp", bufs=3) as wp,
        tc.tile_pool(name="ppg", bufs=1, space="PSUM") as ppg,
        tc.tile_pool(name="ppn", bufs=5, space="PSUM") as ppn,
        tc.tile_pool(name="dp", bufs=1, space="DRAM") as dp,
    ):
        # Warm the ACT LUTs so table loads stay off the critical path.
        warm = iop.tile([1, 1], F32)
        nc.vector.memset(warm[:, :], 0.0)
        for fn in (AF.Exp, AF.Ln, AF.Sigmoid, AF.Tanh):
            nc.scalar.activation(warm[:, :], warm[:, :], fn)

        # LSTM weights first (they gate the whole pipeline), then the packed
        # small inputs: two DMAs instead of five.
        wi_sb = iop.tile([P, KC, GF], LSTM_DT)
        wh_sb = iop.tile([P, KC, GF], LSTM_DT)
        nc.sync.dma_start(wi_sb[:, :, :], io["w_iT"][:, :].rearrange("(a p) g -> p a g", p=P))
        nc.sync.dma_start(wh_sb[:, :, :], io["w_hT"][:, :].rearrange("(a p) g -> p a g", p=P))
        eh_sb = iop.tile([P, 2 * KC], F32)          # e | h0
        nc.sync.dma_start(eh_sb[:, :], io["eh_in"][:, :])
        cb_sb = iop.tile([1, P + GF], F32)          # c0 | gate bias
        nc.sync.dma_start(cb_sb[:, :], io["cb_in"][:, :])
        b_sb = iop.tile([P, TQ], F32)
        nc.sync.dma_start(b_sb[:, :], io["b_full"][:, :])
        if LSTM_DT != F32:
            eh_mm = iop.tile([P, 2 * KC], LSTM_DT)
            nc.vector.tensor_copy(eh_mm[:, :], eh_sb[:, :])
        else:
            eh_mm = eh_sb

        # ---- LSTM gates on one partition: psum_gate[0, g*128+j] ----
        psum_gate = ppg.tile([1, GF], F32)
        for c in range(KC):
            nc.tensor.matmul(psum_gate[:, :], lhsT=eh_mm[:, c:c + 1],
                             rhs=wi_sb[:, c, :], start=(c == 0), stop=False)
        for c in range(KC):
            nc.tensor.matmul(psum_gate[:, :], lhsT=eh_mm[:, KC + c:KC + c + 1],
                             rhs=wh_sb[:, c, :], start=False, stop=(c == KC - 1))
        gsum = iop.tile([1, GF], F32)
        nc.vector.tensor_add(gsum[:, :], psum_gate[:, :], cb_sb[:, P:])
        gact = iop.tile([1, GF], F32)
        nc.scalar.activation(gact[:, 0:2 * P], gsum[:, 0:2 * P], AF.Sigmoid)       # i, f
        nc.scalar.activation(gact[:, 2 * P:3 * P], gsum[:, 2 * P:3 * P], AF.Tanh)  # g
        nc.scalar.activation(gact[:, 3 * P:4 * P], gsum[:, 3 * P:4 * P], AF.Sigmoid)  # o
        fc = iop.tile([1, P], F32)
        nc.vector.tensor_mul(fc[:, :], gact[:, P:2 * P], cb_sb[:, 0:P])
        ig = iop.tile([1, P], F32)
        nc.vector.tensor_mul(ig[:, :], gact[:, 0:P], gact[:, 2 * P:3 * P])
        c_new = iop.tile([1, P], F32)
        nc.vector.tensor_add(c_new[:, :], fc[:, :], ig[:, :])
        tanh_c = iop.tile([1, P], F32)
        nc.scalar.activation(tanh_c[:, :], c_new[:, :], AF.Tanh)
        h_new = iop.tile([1, P], F32)
        nc.vector.tensor_mul(h_new[:, :], gact[:, 3 * P:4 * P], tanh_c[:, :])

        nc.scalar.dma_start(io["h_out"][:, :], h_new[:, :])
        nc.scalar.dma_start(io["c_out"][:, :], c_new[:, :])

        # h' slice to stationary layout [128, 1] via a DRAM round-trip.
        hd = dp.tile([1, P], F32)
        nc.gpsimd.dma_start(hd[:, :], h_new[:, :])
        h_col = iop.tile([P, 1], F32)
        nc.gpsimd.dma_start(h_col[:, :], hd[:, :].rearrange("x p -> p x"))
        # h in column 0 of a [128, 32] stationary tile, zeros elsewhere: each
        # matmul then fills 32 PSUM rows (1 real + 31 zero), so 4 matmuls at
        # col_grp positions {0,32,64,96} initialize the whole [128, 512] tile
        # and it can be staged out with one plain full-tile copy.
        h_pad = iop.tile([P, 32], W_DT)
        nc.vector.memset(h_pad[:, :], 0.0)
        nc.vector.tensor_copy(h_pad[:, 0:1], h_col[:, :])

        # ---- full-vocab partial logits from this core's h-slice ----
        # Block j covers vocab [j*512, (j+1)*512); psum tile t=j//4 holds its
        # 4 blocks on partition rows {0,32,64,96}; the fp16 stage mirrors that.
        # Partials ship in two AllReduce chunks so the first one overlaps the
        # second half of the matvec (each collective has a ~20us floor here).
        stages = [iop.tile([P, TA, NB], AR_DT, name="stageA"),
                  iop.tile([P, NPT - TA, NB], AR_DT, name="stageB")]
        arins = [dp.tile([1, TA * QS * NB], AR_DT, name="arinA"),
                 dp.tile([1, (NPT - TA) * QS * NB], AR_DT, name="arinB")]
        arouts = [dp.tile([1, TA * QS * NB], AR_DT, name="aroutA"),
                  dp.tile([1, (NPT - TA) * QS * NB], AR_DT, name="aroutB")]
        for t in range(NPT):
            stripe = (t * QS) // (SW // NB)
            if (t * QS) % (SW // NB) == 0:
                w0 = stripe * SW
                bw = min(SW, VFULL - w0)
                wt = wp.tile([P, SW], W_DT, tag="wt")
                nc.sync.dma_start(wt[:, :bw], io["w_oT"][:, w0:w0 + bw])
            psum_n = ppn.tile([P, NB], F32, tag="pn")
            for q in range(QS):
                col = (t * QS + q) * NB - stripe * SW
                nc.tensor.matmul(psum_n[q * 32:(q + 1) * 32, :],
                                 lhsT=h_pad[:, :], rhs=wt[:, col:col + NB],
                                 start=True, stop=True,
                                 tile_position=(0, q * 32))
            ch, tt = (0, t) if t < TA else (1, t - TA)
            nc.vector.tensor_copy(stages[ch][:, tt, :], psum_n[:, :])
            if t == TA - 1 or t == NPT - 1:
                nt = TA if ch == 0 else NPT - TA
                for q in range(QS):
                    nc.sync.dma_start(
                        arins[ch][:, :].rearrange("x (t q v) -> x t q v",
                                                  q=QS, v=NB)[:, :, q, :],
                        stages[ch][q * 32:q * 32 + 1, :, :],
                    )
                nc.gpsimd.collective_compute(
                    "AllReduce", ALU.add, replica_groups=RG,
                    ins=[arins[ch].opt()], outs=[arouts[ch].opt()],
                )

        # ---- replicated log_softmax epilogue, chunk c viewed [128, VC/128] ----
        s_parts = iop.tile([P, 2], F32)
        logits_ch = []
        for ch, (tq0, tqn) in enumerate(((0, TQA), (TQA, TQ - TQA))):
            ar_sb = iop.tile([P, tqn], AR_DT, name=f"arsb{ch}")
            nc.sync.dma_start(ar_sb[:, :],
                              arouts[ch][:, :].rearrange("x (p t) -> (x p) t", p=P))
            logits_sb = iop.tile([P, tqn], F32, name=f"lg{ch}")
            nc.vector.tensor_add(logits_sb[:, :], ar_sb[:, :], b_sb[:, tq0:tq0 + tqn])
            # Logits are bounded (~|12|) for this model: exp needs no max-shift.
            exp_sb = iop.tile([P, tqn], F32, name=f"ex{ch}")
            nc.scalar.activation(exp_sb[:, :], logits_sb[:, :], AF.Exp,
                                 accum_out=s_parts[:, ch:ch + 1])
            logits_ch.append(logits_sb)
        s_part = iop.tile([P, 1], F32)
        nc.vector.reduce_sum(s_part[:, :], s_parts[:, :], axis=mybir.AxisListType.X)
        s_red = iop.tile([P, 1], F32)
        nc.gpsimd.partition_all_reduce(s_red[:, :], s_part[:, :], channels=P,
                                       reduce_op=bass_isa_reduce_add())
        logS = iop.tile([P, 1], F32)
        nc.scalar.activation(logS[:, :], s_red[:, :], AF.Ln)
        for ch, (tq0, tqn) in enumerate(((0, TQA), (TQA, TQ - TQA))):
            lp_sb = iop.tile([P, tqn], F32, name=f"lp{ch}")
            nc.vector.tensor_scalar_sub(lp_sb[:, :], logits_ch[ch][:, :], logS[:, :])
            nc.scalar.dma_start(io["lp_out"][:, tq0:tq0 + tqn], lp_sb[:, :])


def bass_isa_reduce_add():
    from concourse import bass_isa
    return bass_isa.ReduceOp.add


_cache = {}


def _build_nc():
    nc = bacc.Bacc("TRN2", target_bir_lowering=False, debug=False, num_devices=NCORES)
    io = {}
    for name, shape, dt in [
        ("w_oT", [P, VFULL], W_DT), ("b_full", [P, TQ], F32),
        ("w_iT", [H, GF], LSTM_DT), ("w_hT", [H, GF], LSTM_DT),
        ("eh_in", [P, 2 * KC], F32), ("cb_in", [1, P + GF], F32),
    ]:
        io[name] = nc.dram_tensor(name, shape, dt, kind="ExternalInput")
    for name, shape in [("lp_out", [P, TQ]), ("h_out", [1, P]), ("c_out", [1, P])]:
        io[name] = nc.dram_tensor(name, shape, F32, kind="ExternalOutput")

    with tile.TileContext(nc) as tc:
        _emit(tc, io)
    nc.compile()
    return nc


def _prep_inputs(inputs):
    emb = np.asarray(inputs["emb"], np.float32)
    tok = int(np.asarray(inputs["input_tok"]).ravel()[0])
    e = emb[tok]
    h0 = np.asarray(inputs["h0"], np.float32).reshape(H)
    c0 = np.asarray(inputs["c0"], np.float32).reshape(H)
    W_ih = np.asarray(inputs["W_ih"], np.float32)
    W_hh = np.asarray(inputs["W_hh"], np.float32)
    b = np.asarray(inputs["b_ih"], np.float32) + np.asarray(inputs["b_hh"], np.float32)
    out_W = np.asarray(inputs["out_W"], np.float32)
    out_b = np.asarray(inputs["out_b"], np.float32)

    w_np = _np_dt[W_DT]
    l_np = _np_dt[LSTM_DT]
    WT = np.ascontiguousarray(out_W.astype(w_np).T)       # [H, V]
    b_flat = np.full((VFULL,), PAD_BIAS, np.float32)
    b_flat[:V] = out_b
    VA = TQA * P
    b_full = np.concatenate([b_flat[:VA].reshape(P, TQA),
                             b_flat[VA:].reshape(P, TQ - TQA)], axis=1)
    b_full = np.ascontiguousarray(b_full)
    eh = np.concatenate([e.reshape(KC, P).T, h0.reshape(KC, P).T], axis=1)
    eh = np.ascontiguousarray(eh)

    in_maps = []
    for k in range(NCORES):
        rows = np.concatenate([np.arange(g * H + k * P, g * H + (k + 1) * P) for g in range(GATES)])
        w_iT = np.ascontiguousarray(W_ih[rows].T.astype(l_np))
        w_hT = np.ascontiguousarray(W_hh[rows].T.astype(l_np))
        w_oT = np.zeros((P, VFULL), w_np)
        w_oT[:, :V] = WT[k * P:(k + 1) * P, :]
        cb = np.concatenate([c0[k * P:(k + 1) * P], b[rows]]).reshape(1, P + GF)
        in_maps.append(dict(w_oT=w_oT, b_full=b_full, w_iT=w_iT, w_hT=w_hT,
                            eh_in=eh, cb_in=np.ascontiguousarray(cb)))
    return in_maps


def _run(inputs, trace=False, **kw):
    if "nc" not in _cache:
        _cache["nc"] = _build_nc()
    nc = _cache["nc"]
    in_maps = _prep_inputs(inputs)
    res = bass_utils.run_bass_kernel_spmd(nc, in_maps, core_ids=list(range(NCORES)),
                                          trace=trace, **kw)
    h_new = np.empty(H, np.float32)
    c_new = np.empty(H, np.float32)
    for k in range(NCORES):
        r = res.results[k]
        h_new[k * P:(k + 1) * P] = r["h_out"].reshape(-1)
        c_new[k * P:(k + 1) * P] = r["c_out"].reshape(-1)
    r0 = res.results[0]["lp_out"]
    lp = np.concatenate([np.ascontiguousarray(r0[:, :TQA]).reshape(-1),
                         np.ascontiguousarray(r0[:, TQA:]).reshape(-1)])[:V]
    out = (lp[None, :], h_new[None, None, :], c_new[None, None, :])
    return out, res


def kernel(**inputs):
    out, _ = _run(inputs)
    return out
